# revision 6
# baseline (speedup 1.0000x reference)
"""Trainium2 Bass kernel for CorrelationNetwork (pairwise-MLP GNN message passing).

Math (per reference):
    f  = agent_features[:, :, :64]                       # [B, N, N]
    Ai = f @ W1[:64];  Aj = f @ W1[64:]                  # [B, N, H]
    h1 = relu(Ai[:, :, None, :] + Aj[:, None, :, :] + b1)
    h2 = relu(h1 @ W2 + b2)
    corr = sigmoid(h2 @ w3 + b3)                         # [B, N, N]
    weights = softmax(mixing_weights)

Device strategy (8 cores, batch-sharded 128 -> 16 per core), all on-chip:
  * fT via PE transpose; layer-1 matmuls with b1 folded via an appended
    ones-row (K=65).
  * The pairwise broadcast add (outer sum) is computed ON THE PE as
    M_b.T @ R where M_b stacks [Ai'; Aj] on partitions (i rows 0-63,
    j rows 64-127) and R is a constant 0/1 selection matrix [128, 4096].
  * h1/h2 stay feature-major [H=128 partitions, cols=(i,j)]; relu
    evacuations PSUM->SBUF are split across ScalarE/VectorE.
  * w3 reduction runs as matmuls with the DATA as the stationary operand
    (lhsT = h2relu tile [128h, 128cols], rhs = w3 [128,1]) so z lands
    across 128 PSUM partitions -> cheap sigmoid + DMA.
  * sigmoid(x) = 0.5*tanh(0.5 x) + 0.5 so every ACT func (relu/tanh/exp)
    lives in one table set.
"""

import sys

sys.path.insert(0, "/opt/trn_rl_repo")

from contextlib import ExitStack

import ml_dtypes
import numpy as np

import concourse.bass as bass
import concourse.tile as tile
from concourse import bacc, mybir
from concourse.bass_utils import run_bass_kernel_spmd

F32 = mybir.dt.float32
BF16 = mybir.dt.bfloat16
AF = mybir.ActivationFunctionType
ALU = mybir.AluOpType

NCORES = 8
B, N, F, H = 128, 64, 128, 128
BLOC = B // NCORES  # 16 batches per core
COLS = N * N  # 4096 pairwise columns per batch
ZROUND = 4  # batches per z-evacuation round


def _np_bf16(x):
    return np.asarray(x, dtype=ml_dtypes.bfloat16)


def _build_rsel():
    """R[k, 64*i+j] = (k<64 and k==i) or (k>=64 and k-64==j), bf16."""
    R = np.zeros((128, COLS), dtype=np.float32)
    for i in range(N):
        R[i, i * N:(i + 1) * N] = 1.0
    for j in range(N):
        R[64 + j, j::N] = 1.0
    return _np_bf16(R)


def _build_kernel(ctx: ExitStack, tc: tile.TileContext, io: dict):
    nc = tc.nc

    const = ctx.enter_context(tc.tile_pool(name="const", bufs=1))
    fin = ctx.enter_context(tc.tile_pool(name="fin", bufs=1))
    ftp = ctx.enter_context(tc.tile_pool(name="ftp", bufs=1))
    msb = ctx.enter_context(tc.tile_pool(name="msb", bufs=1))
    h1r = ctx.enter_context(tc.tile_pool(name="h1r", bufs=2))
    h2r = ctx.enter_context(tc.tile_pool(name="h2r", bufs=2))
    zsb = ctx.enter_context(tc.tile_pool(name="zsb", bufs=2))
    tiny = ctx.enter_context(tc.tile_pool(name="tiny", bufs=1))
    ps_h1 = ctx.enter_context(tc.tile_pool(name="ps_h1", bufs=2, space="PSUM"))
    ps_h2 = ctx.enter_context(tc.tile_pool(name="ps_h2", bufs=2, space="PSUM"))
    ps_z = ctx.enter_context(tc.tile_pool(name="ps_z", bufs=2, space="PSUM"))

    # ---------------- constants / weights to SBUF ----------------
    identb = const.tile([128, 128], BF16, tag="identb")
    nc.sync.dma_start(identb[:], io["identb"])
    identf = const.tile([128, 128], F32, tag="identf")
    nc.sync.dma_start(identf[:], io["identf"])
    rsel = const.tile([128, COLS], BF16, tag="rsel")
    nc.sync.dma_start(rsel[:], io["rsel"])

    # W1a/W1b with bias row (row 64 = b1 for the i-half, 0 for the j-half)
    w1a = const.tile([65, H], BF16, tag="w1a")
    nc.gpsimd.dma_start(w1a[0:64, :], io["w1"][0:64, :])
    nc.gpsimd.dma_start(w1a[64:65, :], io["b1v"].rearrange("(o h) -> o h", o=1))
    w1b = const.tile([65, H], BF16, tag="w1b")
    nc.gpsimd.dma_start(w1b[0:64, :], io["w1"][64:128, :])
    nc.vector.memset(w1b[64:65, :], 0.0)

    w2 = const.tile([H, H], BF16, tag="w2")
    nc.gpsimd.dma_start(w2[:], io["w2"])
    w3c = const.tile([H, 1], BF16, tag="w3c")
    nc.gpsimd.dma_start(w3c[:], io["w3v"].rearrange("(h o) -> h o", o=1))
    b2c = const.tile([H, 1], F32, tag="b2c")
    nc.sync.dma_start(b2c[:], io["b2v"].rearrange("(h o) -> h o", o=1))

    # b3 replicated across partitions via a K=1 matmul with a ones row,
    # then pre-scaled by 0.5 for the tanh-based sigmoid.
    onesr = const.tile([1, 128], BF16, tag="onesr")
    nc.vector.memset(onesr[:], 1.0)
    b3s = const.tile([1, 1], BF16, tag="b3s")
    nc.gpsimd.dma_start(b3s[:], io["b3v"].rearrange("(o u) -> o u", o=1))
    ps_rep = ps_z.tile([128, 128], F32, tag="zz")
    nc.tensor.matmul(ps_rep[:, 0:1], onesr[:], b3s[:], start=True, stop=True)
    b3h = const.tile([128, 1], F32, tag="b3h")
    nc.vector.tensor_scalar_mul(b3h[:], ps_rep[:, 0:1], 0.5)

    # ---------------- softmax(mixing) -> weights_out ----------------
    mix = tiny.tile([1, N], F32, tag="mix")
    nc.sync.dma_start(mix[:], io["mix"].rearrange("(o n) -> o n", o=1))
    mmax = tiny.tile([1, 1], F32, tag="mmax")
    nc.vector.tensor_reduce(mmax[:], mix[:], axis=mybir.AxisListType.X, op=ALU.max)
    mneg = tiny.tile([1, 1], F32, tag="mneg")
    nc.vector.tensor_scalar_mul(mneg[:], mmax[:], -1.0)
    mexp = tiny.tile([1, N], F32, tag="mexp")
    nc.scalar.activation(mexp[:], mix[:], AF.Exp, bias=mneg[:], scale=1.0)
    msum = tiny.tile([1, 1], F32, tag="msum")
    nc.vector.tensor_reduce(msum[:], mexp[:], axis=mybir.AxisListType.X, op=ALU.add)
    mrec = tiny.tile([1, 1], F32, tag="mrec")
    nc.vector.reciprocal(mrec[:], msum[:])
    mout = tiny.tile([1, N], F32, tag="mout")
    nc.vector.tensor_scalar_mul(mout[:], mexp[:], mrec[:])
    nc.sync.dma_start(io["weights_out"], mout[:])

    # ---------------- load f (cast to bf16) and transpose ----------------
    # af is [1024, 128] fp32 in DRAM; SBUF tile [128, t*128+c] = af[128t+p, c]
    f_sb = fin.tile([128, 8 * 128], BF16, tag="f_sb")
    nc.gpsimd.dma_start(
        f_sb[:].rearrange("p (t c) -> p t c", c=128),
        io["af"].rearrange("(t p) c -> p t c", p=128),
    )

    ft = ftp.tile([128, 1024], BF16, tag="ft")  # rows 0-63: fT, row 64: ones
    nc.vector.memset(ft[64:65, :], 1.0)
    for t in range(8):
        pst = ps_z.tile([128, 128], BF16, tag="zz")
        nc.tensor.transpose(pst[0:64, :], f_sb[:, t * 128:t * 128 + 64], identb[:])
        nc.vector.tensor_copy(ft[0:64, t * 128:(t + 1) * 128], pst[0:64, :])

    # ---------------- layer 1: M_b = [Ai'+b1 ; Aj] per batch ----------------
    m_all = msb.tile([128, BLOC * H], BF16, tag="m_all")
    for b in range(BLOC):
        psa = ps_z.tile([128, H], F32, tag="zz")
        lhs = ft[0:65, b * N:(b + 1) * N]
        nc.tensor.matmul(psa[0:64, :], lhs, w1a[:], start=True, stop=True)
        nc.tensor.matmul(
            psa[64:128, :], lhs, w1b[:], start=True, stop=True,
            tile_position=(0, 64),
        )
        nc.vector.tensor_copy(m_all[:, b * H:(b + 1) * H], psa[:])

    # ---------------- main pipeline over batches ----------------
    corr = io["corr_out"]  # [4, 128, 128] fp32 DRAM
    zacc = None
    evac_idx = 0  # rotates ACT/DVE assignment

    for b in range(BLOC):
        if b % ZROUND == 0:
            zacc = ps_z.tile([128, 128], F32, tag="zz")
        mb = m_all[:, b * H:(b + 1) * H]

        # outer sum -> h1 tiles (4 chunks of 1024 cols)
        h1tiles = []
        for c in range(4):
            ph1 = ps_h1.tile([128, 1024], F32, tag="ph1")
            for s in range(2):
                lo = c * 1024 + s * 512
                nc.tensor.matmul(
                    ph1[:, s * 512:(s + 1) * 512], mb, rsel[:, lo:lo + 512],
                    start=True, stop=True,
                )
            h1t = h1r.tile([128, 1024], BF16, tag="h1t")
            if evac_idx % 2 == 0:
                nc.scalar.activation(h1t[:], ph1[:], AF.Relu, bias=0.0, scale=1.0)
            else:
                nc.vector.tensor_scalar(h1t[:], ph1[:], 0.0, None, op0=ALU.max)
            evac_idx += 1
            h1tiles.append(h1t)

        # main matmul + relu(+b2) + w3 reduction, chunk by chunk (512 cols)
        for c in range(8):
            ph2 = ps_h2.tile([128, 512], F32, tag="ph2")
            rhs = h1tiles[c // 2][:, (c % 2) * 512:(c % 2 + 1) * 512]
            nc.tensor.matmul(ph2[:], w2[:], rhs, start=True, stop=True)
            h2t = h2r.tile([128, 512], BF16, tag="h2t")
            if evac_idx % 8 in (0, 2, 3, 5, 6):
                nc.scalar.activation(h2t[:], ph2[:], AF.Relu, bias=b2c[:], scale=1.0)
            else:
                nc.vector.tensor_scalar(
                    h2t[:], ph2[:], b2c[:], 0.0, op0=ALU.add, op1=ALU.max
                )
            evac_idx += 1
            for m in range(4):
                zc = 32 * (b % ZROUND) + c * 4 + m
                nc.tensor.matmul(
                    zacc[:, zc:zc + 1], h2t[:, m * 128:(m + 1) * 128], w3c[:],
                    start=True, stop=True,
                )

        # z round: copy -> transpose -> sigmoid((tanh trick)) -> DMA out
        if b % ZROUND == ZROUND - 1:
            r = b // ZROUND
            z_sb = zsb.tile([128, 128], F32, tag="z_sb")
            nc.vector.tensor_copy(z_sb[:], zacc[:])
            zT = ps_z.tile([128, 128], F32, tag="zz")
            nc.tensor.transpose(zT[:], z_sb[:], identf[:])
            zo = zsb.tile([128, 128], F32, tag="zo")
            nc.scalar.activation(zo[:], zT[:], AF.Tanh, bias=b3h[:], scale=0.5)
            zf = zsb.tile([128, 128], F32, tag="zf")
            nc.vector.tensor_scalar(zf[:], zo[:], 0.5, 0.5, op0=ALU.mult, op1=ALU.add)
            nc.sync.dma_start(corr[r], zf[:])


_CACHE = {}


def _program():
    if "nc" in _CACHE:
        return _CACHE["nc"]
    nc = bacc.Bacc(
        "TRN2", target_bir_lowering=False, debug=False, num_devices=NCORES
    )
    io = {
        "af": nc.dram_tensor("af", [BLOC * N, F], F32, kind="ExternalInput").ap(),
        "w1": nc.dram_tensor("w1", [F, H], F32, kind="ExternalInput").ap(),
        "b1v": nc.dram_tensor("b1v", [H], F32, kind="ExternalInput").ap(),
        "w2": nc.dram_tensor("w2", [H, H], F32, kind="ExternalInput").ap(),
        "b2v": nc.dram_tensor("b2v", [H], F32, kind="ExternalInput").ap(),
        "w3v": nc.dram_tensor("w3v", [H], F32, kind="ExternalInput").ap(),
        "b3v": nc.dram_tensor("b3v", [1], F32, kind="ExternalInput").ap(),
        "mix": nc.dram_tensor("mix", [N], F32, kind="ExternalInput").ap(),
        "rsel": nc.dram_tensor("rsel", [128, COLS], BF16, kind="ExternalInput").ap(),
        "identb": nc.dram_tensor("identb", [128, 128], BF16, kind="ExternalInput").ap(),
        "identf": nc.dram_tensor("identf", [128, 128], F32, kind="ExternalInput").ap(),
        "corr_out": nc.dram_tensor(
            "corr_out", [BLOC // ZROUND, 128, 128], F32, kind="ExternalOutput"
        ).ap(),
        "weights_out": nc.dram_tensor(
            "weights_out", [1, N], F32, kind="ExternalOutput"
        ).ap(),
    }
    with tile.TileContext(nc) as tc:
        with ExitStack() as ctx:
            _build_kernel(ctx, tc, io)
    nc.compile()
    _CACHE["nc"] = nc
    return nc


def _in_maps(agent_features, W1, b1, W2, b2, w3, b3, mixing_weights):
    ident = np.eye(128, dtype=np.float32)
    shared = {
        "w1": np.ascontiguousarray(np.asarray(W1, dtype=np.float32)),
        "b1v": np.ascontiguousarray(np.asarray(b1, dtype=np.float32)),
        "w2": np.ascontiguousarray(np.asarray(W2, dtype=np.float32)),
        "b2v": np.ascontiguousarray(np.asarray(b2, dtype=np.float32)),
        "w3v": np.ascontiguousarray(np.asarray(w3, dtype=np.float32)),
        "b3v": np.ascontiguousarray(np.asarray(b3, dtype=np.float32)),
        "mix": np.ascontiguousarray(np.asarray(mixing_weights, dtype=np.float32)),
        "rsel": _build_rsel(),
        "identb": _np_bf16(ident),
        "identf": ident,
    }
    af = np.ascontiguousarray(np.asarray(agent_features, dtype=np.float32))
    return [
        dict(shared, af=af[c * BLOC:(c + 1) * BLOC].reshape(BLOC * N, F))
        for c in range(NCORES)
    ]


def kernel(agent_features, W1, b1, W2, b2, w3, b3, mixing_weights, **kw):
    nc = _program()

    in_maps = _in_maps(
        agent_features, W1, b1, W2, b2, w3, b3, mixing_weights
    )
    res = run_bass_kernel_spmd(nc, in_maps, core_ids=list(range(NCORES)))
    corr = np.concatenate(
        [res.results[c]["corr_out"].reshape(BLOC, N, N) for c in range(NCORES)],
        axis=0,
    ).astype(np.float32)
    weights = res.results[0]["weights_out"].reshape(N).astype(np.float32)
    return corr, weights
